# revision 19
# baseline (speedup 1.0000x reference)
"""Conv1d (B=32, C_in=C_out=64, L=16384, K=3, VALID) on 8 trn2 cores.

Strategy: data-parallel over batch (4 batches/core). Each core views its
shard as 2 "pairs" of batches stacked into 128 partitions. The conv is
3 PSUM-accumulated matmuls (one per tap) against a block-diagonal
weight lhsT [128, 128] = diag(W_k^T, W_k^T), so one matmul computes two
batches at full 128-partition PE utilization. Accumulation is fp32 in
PSUM; I/O streams are fp16 to halve HBM traffic (the memory roofline).
Bias is fused into the PSUM->SBUF copy. Shapes hardcoded from the spec.
"""

import os

import numpy as np

from concourse import bacc, bass, mybir, tile
from concourse.bass_utils import run_bass_kernel_spmd

B, C, L, K = 32, 64, 16384, 3
LOUT = L - K + 1  # 16382
NCORES = 8
BPC = B // NCORES  # 4 batches per core
PAIRS = BPC // 2  # 2 stacked pairs per core
P = 128  # partitions (2 x C)
NJ = int(os.environ.get("CONV_NJ", "512"))  # PSUM inner chunk

F32 = mybir.dt.float32

# precision mode: f16 I/O (default, ~3e-4 rel err) or f32r / f32
MODE = os.environ.get("CONV_MODE", "f16")
CH = int(os.environ.get("CONV_CH", "4096" if MODE == "f16" else "2048"))
IBUFS = int(os.environ.get("CONV_IBUFS", "8"))
OBUFS = int(os.environ.get("CONV_OBUFS", "6"))
WARMUP = int(os.environ.get("CONV_WARMUP", "4"))
NARROW = int(os.environ.get("CONV_NARROW", "0"))
# NOTE: the Scalar HWDGE queue (q10) moves bulk data in ~1KB packets on
# one engine (~3x slower than Sync HWDGE / GpSimd SWDGE) — do not use it.
# Output chunks go to GpSimd SWDGE, except the last SYNC_TAIL chunks
# which go on the Sync HWDGE queue (idle once inputs are issued; fast
# first-byte + cheap drain shortens the end-of-kernel tail).
SYNC_TAIL = int(os.environ.get("CONV_SYNC_TAIL", "6"))
# queue for weights/bias and which early input chunks ride SWDGE in
# parallel with Sync (comma list of chunk indices; empty disables)
WQ = os.environ.get("CONV_WQ", "gpsimd")
SPLIT_IN = [int(v) for v in os.environ.get("CONV_SPLIT_IN", "").split(",")
            if v]
RAMP = [int(v) for v in os.environ.get("CONV_RAMP", "1024,1024,2048").split(",") if v]
TAIL = [int(v) for v in os.environ.get("CONV_TAIL", "1024,512,256").split(",") if v]

_NC_CACHE = []


def _io_dtypes():
    if MODE == "f16":
        return mybir.dt.float16, mybir.dt.float16, np.float16
    if MODE == "f32r":
        return mybir.dt.float32r, F32, np.float32
    return F32, F32, np.float32


def _build_nc():
    FIN, FOUT, _ = _io_dtypes()
    nc = bacc.Bacc("TRN2", target_bir_lowering=False, debug=False,
                   num_devices=NCORES)

    x2 = nc.dram_tensor("x2", [PAIRS, P, L], FIN, kind="ExternalInput")
    wT = nc.dram_tensor("wT", [P, K, P], FIN, kind="ExternalInput")
    b2 = nc.dram_tensor("b2", [P, 1], F32, kind="ExternalInput")
    y2 = nc.dram_tensor("y2", [PAIRS, P, LOUT], FOUT, kind="ExternalOutput")

    with tile.TileContext(nc) as tc:
        with (
            tc.tile_pool(name="const", bufs=1) as const_pool,
            tc.tile_pool(name="inp", bufs=IBUFS) as inp_pool,
            tc.tile_pool(name="outp", bufs=OBUFS) as outp_pool,
            tc.tile_pool(name="psum", bufs=8 * 512 // NJ,
                         space=bass.MemorySpace.PSUM) as psum_pool,
        ):
            # weights ride GpSimd SWDGE (idle until outputs start) so the
            # Sync queue's first descriptor is input chunk 0; both first
            # transfers overlap their pipeline-fill latencies.
            weng = nc.gpsimd if WQ == "gpsimd" else nc.sync
            w = const_pool.tile([P, K, P], FIN)
            weng.dma_start(out=w[:], in_=wT[:])
            bias = const_pool.tile([P, 1], F32)
            weng.dma_start(out=bias[:], in_=b2[:])

            # HAM warm-up: dummy matmuls on zeroed SBUF while the first
            # input DMA is in flight, so the PE clock gate is near 8/8
            # (2.4 GHz) when real work arrives instead of ramping through
            # the first ~3.4us of it. memset on GpSimd (idle) so warm-up
            # isn't gated behind DVE start-up.
            if WARMUP:
                wz = const_pool.tile([P, 512], FIN)
                nc.gpsimd.memset(wz[:], 0.0)
                # wide matmuls ramp the clock; a tail of narrow ones keeps
                # the PE active until the first input chunk lands without
                # a wide op blocking the first real matmul.
                for i in range(WARMUP + NARROW):
                    nw = 512 if i < WARMUP else 128
                    wp = psum_pool.tile([P, NJ], F32, tag="acc",
                                        name=f"warm{i}")
                    nc.tensor.matmul(wp[:, :nw], wz[:, :P], wz[:, :nw],
                                     start=True, stop=True)

            # Input DMAs issue from Sync (HWDGE, fast first-byte) so the
            # pipeline fills immediately; output DMAs go to GpSimd (SWDGE)
            # and/or Scalar (HWDGE) per OUTQ. Chunk sizes are shaped: small
            # first chunks so compute starts early (and rides the clock
            # ramp), small last chunks so the compute-gated tail after the
            # final input is short.
            ramp = RAMP
            tail_small = TAIL
            rest = LOUT - sum(ramp)
            body = [CH] * (rest // CH)
            last = rest - sum(body)
            rest1 = LOUT - sum(tail_small)
            body1 = [CH] * (rest1 // CH)
            last1 = rest1 - sum(body1)
            chunk_lists = {
                0: ramp + body + [last],
                1: body1 + [last1] + tail_small,
            }
            nchunks = sum(len(v) for v in chunk_lists.values())
            ci = 0
            for p in range(PAIRS):
                l0 = 0
                for n in chunk_lists[p % 2]:
                    nin = n + K - 1  # l0 + nin <= L always (LOUT = L-2)
                    it = inp_pool.tile([P, CH + K - 1], FIN, tag="in")
                    ieng = nc.gpsimd if ci in SPLIT_IN else nc.sync
                    ieng.dma_start(out=it[:, :nin],
                                   in_=x2[p, :, l0:l0 + nin])
                    ot = outp_pool.tile([P, CH], FOUT, tag="out")
                    for j0 in range(0, n, NJ):
                        nj = min(NJ, n - j0)
                        pt = psum_pool.tile([P, NJ], F32, tag="acc")
                        for k in range(K):
                            nc.tensor.matmul(
                                pt[:, :nj],
                                w[:, k, :],
                                it[:, j0 + k:j0 + k + nj],
                                start=(k == 0),
                                stop=(k == K - 1),
                            )
                        # psum -> sbuf with fused bias add, split across
                        # ACT and DVE so the bank frees twice as fast
                        h = nj // 2
                        nc.scalar.add(ot[:, j0:j0 + h], pt[:, :h],
                                      add=bias[:, 0:1])
                        nc.vector.tensor_scalar_add(ot[:, j0 + h:j0 + nj],
                                                    pt[:, h:nj],
                                                    bias[:, 0:1])
                    oeng = nc.sync if ci >= nchunks - SYNC_TAIL else nc.gpsimd
                    oeng.dma_start(out=y2[p, :, l0:l0 + n],
                                   in_=ot[:, :n])
                    l0 += n
                    ci += 1

    nc.compile()
    return nc


def _get_nc():
    if not _NC_CACHE:
        _NC_CACHE.append(_build_nc())
    return _NC_CACHE[0]


def _prep_weights(weight, bias, np_in):
    wT = np.zeros((P, K, P), np.float32)
    for k in range(K):
        wtk = np.ascontiguousarray(weight[:, :, k].T)  # [C_in, C_out]
        wT[0:C, k, 0:C] = wtk
        wT[C:P, k, C:P] = wtk
    b2 = np.concatenate([bias, bias]).reshape(P, 1).astype(np.float32)
    return wT.astype(np_in), b2


def kernel(x, weight, bias, _want_results=False, **run_kwargs):
    x = np.asarray(x, np.float32)
    weight = np.asarray(weight, np.float32)
    bias = np.asarray(bias, np.float32)
    _, _, np_in = _io_dtypes()
    nc = _get_nc()
    wT, b2 = _prep_weights(weight, bias, np_in)
    in_maps = [
        {
            "x2": np.ascontiguousarray(
                x[BPC * i:BPC * (i + 1)].reshape(PAIRS, P, L)).astype(
                    np_in, copy=False),
            "wT": wT,
            "b2": b2,
        }
        for i in range(NCORES)
    ]
    res = run_bass_kernel_spmd(nc, in_maps, list(range(NCORES)), **run_kwargs)
    out = np.concatenate(
        [
            res.results[i]["y2"].astype(np.float32).reshape(BPC, C, LOUT)
            for i in range(NCORES)
        ],
        axis=0,
    )
    if _want_results:
        return out, res
    return out



# revision 20
# speedup vs baseline: 1.0064x; 1.0064x over previous
"""Conv1d (B=32, C_in=C_out=64, L=16384, K=3, VALID) on 8 trn2 cores.

Strategy: data-parallel over batch (4 batches/core). Each core views its
shard as 2 "pairs" of batches stacked into 128 partitions. The conv is
3 PSUM-accumulated matmuls (one per tap) against a block-diagonal
weight lhsT [128, 128] = diag(W_k^T, W_k^T), so one matmul computes two
batches at full 128-partition PE utilization. Accumulation is fp32 in
PSUM; I/O streams are fp16 to halve HBM traffic (the memory roofline).
Bias is fused into the PSUM->SBUF copy. Shapes hardcoded from the spec.

Pipeline findings (perfetto, see git/session notes):
- exec ~= 7us NEFF preamble + ~4.5us (first chunk DMA latency + HAM
  clock ramp, hidden behind warm-up matmuls) + ~43.5us matmul stream
  (192 MMs of 512 cols at ~220ns issue-to-issue ~= f16 PE peak for
  this block-diag scheme) + ~4.5us output tail + ~3.3us teardown.
- The MM stream is the floor: fp8 fails accuracy (3.2e-2 > 2e-2 even
  e4m3; hi+lo splits 2.1-2.4e-2), Winograd moves work to DVE (4x
  slower/col), matmul free dim is ISA-capped at 512.
- Weights ride the SWDGE queue (idle early) so Sync's first descriptor
  is input chunk 0; both first-transfer latencies overlap. Inputs must
  ALL stay on Sync HWDGE - SWDGE inputs starve the PE (+8us). The
  Scalar HWDGE queue (q10) is ~1KB-packet/single-engine slow - unused.
- Last SYNC_TAIL output chunks ride Sync (idle at the end, fast drain);
  earlier outputs ride SWDGE. Tail chunks shrink to 256 cols so the
  last compute->DMA->drain hop is short.
"""

import os

import numpy as np

from concourse import bacc, bass, mybir, tile
from concourse.bass_utils import run_bass_kernel_spmd

B, C, L, K = 32, 64, 16384, 3
LOUT = L - K + 1  # 16382
NCORES = 8
BPC = B // NCORES  # 4 batches per core
PAIRS = BPC // 2  # 2 stacked pairs per core
P = 128  # partitions (2 x C)
NJ = int(os.environ.get("CONV_NJ", "512"))  # PSUM inner chunk

F32 = mybir.dt.float32

# precision mode: f16 I/O (default, ~3e-4 rel err) or f32r / f32
MODE = os.environ.get("CONV_MODE", "f16")
CH = int(os.environ.get("CONV_CH", "4096" if MODE == "f16" else "2048"))
IBUFS = int(os.environ.get("CONV_IBUFS", "8"))
OBUFS = int(os.environ.get("CONV_OBUFS", "6"))
WARMUP = int(os.environ.get("CONV_WARMUP", "4"))
NARROW = int(os.environ.get("CONV_NARROW", "0"))
# NOTE: the Scalar HWDGE queue (q10) moves bulk data in ~1KB packets on
# one engine (~3x slower than Sync HWDGE / GpSimd SWDGE) — do not use it.
# Output chunks go to GpSimd SWDGE, except the last SYNC_TAIL chunks
# which go on the Sync HWDGE queue (idle once inputs are issued; fast
# first-byte + cheap drain shortens the end-of-kernel tail).
SYNC_TAIL = int(os.environ.get("CONV_SYNC_TAIL", "6"))
# queue for weights/bias and which early input chunks ride SWDGE in
# parallel with Sync (comma list of chunk indices; empty disables)
WQ = os.environ.get("CONV_WQ", "gpsimd")
SPLIT_IN = [int(v) for v in os.environ.get("CONV_SPLIT_IN", "").split(",")
            if v]
RAMP = [int(v) for v in os.environ.get("CONV_RAMP", "1024,1024,2048").split(",") if v]
TAIL = [int(v) for v in os.environ.get("CONV_TAIL", "1024,512,256").split(",") if v]

_NC_CACHE = []


def _io_dtypes():
    if MODE == "f16":
        return mybir.dt.float16, mybir.dt.float16, np.float16
    if MODE == "f32r":
        return mybir.dt.float32r, F32, np.float32
    return F32, F32, np.float32


def _build_nc():
    FIN, FOUT, _ = _io_dtypes()
    nc = bacc.Bacc("TRN2", target_bir_lowering=False, debug=False,
                   num_devices=NCORES)

    x2 = nc.dram_tensor("x2", [PAIRS, P, L], FIN, kind="ExternalInput")
    wT = nc.dram_tensor("wT", [P, K, P], FIN, kind="ExternalInput")
    b2 = nc.dram_tensor("b2", [P, 1], F32, kind="ExternalInput")
    y2 = nc.dram_tensor("y2", [PAIRS, P, LOUT], FOUT, kind="ExternalOutput")

    with tile.TileContext(nc) as tc:
        with (
            tc.tile_pool(name="const", bufs=1) as const_pool,
            tc.tile_pool(name="inp", bufs=IBUFS) as inp_pool,
            tc.tile_pool(name="outp", bufs=OBUFS) as outp_pool,
            tc.tile_pool(name="psum", bufs=8 * 512 // NJ,
                         space=bass.MemorySpace.PSUM) as psum_pool,
        ):
            # weights ride GpSimd SWDGE (idle until outputs start) so the
            # Sync queue's first descriptor is input chunk 0; both first
            # transfers overlap their pipeline-fill latencies.
            weng = nc.gpsimd if WQ == "gpsimd" else nc.sync
            w = const_pool.tile([P, K, P], FIN)
            weng.dma_start(out=w[:], in_=wT[:])
            bias = const_pool.tile([P, 1], F32)
            weng.dma_start(out=bias[:], in_=b2[:])

            # HAM warm-up: dummy matmuls on zeroed SBUF while the first
            # input DMA is in flight, so the PE clock gate is near 8/8
            # (2.4 GHz) when real work arrives instead of ramping through
            # the first ~3.4us of it. memset on GpSimd (idle) so warm-up
            # isn't gated behind DVE start-up.
            if WARMUP:
                wz = const_pool.tile([P, 512], FIN)
                nc.gpsimd.memset(wz[:], 0.0)
                # wide matmuls ramp the clock; a tail of narrow ones keeps
                # the PE active until the first input chunk lands without
                # a wide op blocking the first real matmul.
                for i in range(WARMUP + NARROW):
                    nw = 512 if i < WARMUP else 128
                    wp = psum_pool.tile([P, NJ], F32, tag="acc",
                                        name=f"warm{i}")
                    nc.tensor.matmul(wp[:, :nw], wz[:, :P], wz[:, :nw],
                                     start=True, stop=True)

            # Input DMAs issue from Sync (HWDGE, fast first-byte) so the
            # pipeline fills immediately; output DMAs go to GpSimd (SWDGE)
            # and/or Scalar (HWDGE) per OUTQ. Chunk sizes are shaped: small
            # first chunks so compute starts early (and rides the clock
            # ramp), small last chunks so the compute-gated tail after the
            # final input is short.
            ramp = RAMP
            tail_small = TAIL
            rest = LOUT - sum(ramp)
            body = [CH] * (rest // CH)
            last = rest - sum(body)
            rest1 = LOUT - sum(tail_small)
            body1 = [CH] * (rest1 // CH)
            last1 = rest1 - sum(body1)
            chunk_lists = {
                0: ramp + body + [last],
                1: body1 + [last1] + tail_small,
            }
            nchunks = sum(len(v) for v in chunk_lists.values())
            ci = 0
            for p in range(PAIRS):
                l0 = 0
                for n in chunk_lists[p % 2]:
                    nin = n + K - 1  # l0 + nin <= L always (LOUT = L-2)
                    it = inp_pool.tile([P, CH + K - 1], FIN, tag="in")
                    ieng = nc.gpsimd if ci in SPLIT_IN else nc.sync
                    ieng.dma_start(out=it[:, :nin],
                                   in_=x2[p, :, l0:l0 + nin])
                    ot = outp_pool.tile([P, CH], FOUT, tag="out")
                    for j0 in range(0, n, NJ):
                        nj = min(NJ, n - j0)
                        pt = psum_pool.tile([P, NJ], F32, tag="acc")
                        for k in range(K):
                            nc.tensor.matmul(
                                pt[:, :nj],
                                w[:, k, :],
                                it[:, j0 + k:j0 + k + nj],
                                start=(k == 0),
                                stop=(k == K - 1),
                            )
                        # psum -> sbuf with fused bias add, split across
                        # ACT and DVE so the bank frees twice as fast
                        h = nj // 2
                        nc.scalar.add(ot[:, j0:j0 + h], pt[:, :h],
                                      add=bias[:, 0:1])
                        nc.vector.tensor_scalar_add(ot[:, j0 + h:j0 + nj],
                                                    pt[:, h:nj],
                                                    bias[:, 0:1])
                    oeng = nc.sync if ci >= nchunks - SYNC_TAIL else nc.gpsimd
                    oeng.dma_start(out=y2[p, :, l0:l0 + n],
                                   in_=ot[:, :n])
                    l0 += n
                    ci += 1

    nc.compile()
    return nc


def _get_nc():
    if not _NC_CACHE:
        _NC_CACHE.append(_build_nc())
    return _NC_CACHE[0]


def _prep_weights(weight, bias, np_in):
    wT = np.zeros((P, K, P), np.float32)
    for k in range(K):
        wtk = np.ascontiguousarray(weight[:, :, k].T)  # [C_in, C_out]
        wT[0:C, k, 0:C] = wtk
        wT[C:P, k, C:P] = wtk
    b2 = np.concatenate([bias, bias]).reshape(P, 1).astype(np.float32)
    return wT.astype(np_in), b2


def kernel(x, weight, bias, _want_results=False, **run_kwargs):
    x = np.asarray(x, np.float32)
    weight = np.asarray(weight, np.float32)
    bias = np.asarray(bias, np.float32)
    _, _, np_in = _io_dtypes()
    nc = _get_nc()
    wT, b2 = _prep_weights(weight, bias, np_in)
    in_maps = [
        {
            "x2": np.ascontiguousarray(
                x[BPC * i:BPC * (i + 1)].reshape(PAIRS, P, L)).astype(
                    np_in, copy=False),
            "wT": wT,
            "b2": b2,
        }
        for i in range(NCORES)
    ]
    res = run_bass_kernel_spmd(nc, in_maps, list(range(NCORES)), **run_kwargs)
    out = np.concatenate(
        [
            res.results[i]["y2"].astype(np.float32).reshape(BPC, C, LOUT)
            for i in range(NCORES)
        ],
        axis=0,
    )
    if _want_results:
        return out, res
    return out



# revision 21
# speedup vs baseline: 1.0419x; 1.0352x over previous
"""Conv1d (B=32, C_in=C_out=64, L=16384, K=3, VALID) on 8 trn2 cores.

Strategy: data-parallel over batch (4 batches/core), POLYPHASE compute.
The host deinterleaves each batch's length axis into even/odd streams,
giving a [128 = (parity x 64 ch), L/2] SBUF view. One rhs column then
carries TWO input samples, and two PSUM-accumulated matmuls against
dense-ish [128,128] weights (A for column t, B for column t+1) produce
TWO output samples per column:
  out[(p=0)*64+co, t] = y[co, 2t]   = w0 xe[t] + w1 xo[t] + w2 xe[t+1]
  out[(p=1)*64+co, t] = y[co, 2t+1] = w0 xo[t] + w1 xe[t+1] + w2 xo[t+1]
  A[e*64+ci, p*64+co]: (0,0)=w0T (1,0)=w1T (1,1)=w0T;  (0,1)=0
  B[e*64+ci, p*64+co]: (0,0)=w2T (0,1)=w1T (1,1)=w2T;  (1,0)=0
That is 1.0 PE cycle per output sample per batch (75% array util) vs
1.5 (50%) for the tap-per-matmul block-diagonal scheme — the matmul
stream drops from ~43.5us to ~29us and the kernel becomes DMA-paced.
fp32 PSUM accumulation; f16 I/O halves HBM traffic (memory roofline).
Bias is fused into the PSUM->SBUF copy, split across ACT and DVE.
Host re-interleaves the [128, L/2] output back to [64, LOUT].

Queue/pipeline findings baked in (perfetto-driven, see session notes):
- ~7us NEFF preamble + ~3.3us teardown are fixed; DMA+PE clocks ramp
  (HAM) over the first ~10us — warm-up matmuls ride it out.
- Inputs must all stay on Sync HWDGE (SWDGE inputs starve the PE);
  weights ride SWDGE (idle early) so Sync's first descriptor is input
  chunk 0; Scalar HWDGE (q10) is ~1KB-packet slow — never used.
- Outputs ride SWDGE except the last SYNC_TAIL chunks on Sync (idle by
  then, fast drain); tail chunks shrink so the last compute->DMA->drain
  hop is short.
"""

import os

import numpy as np

from concourse import bacc, bass, mybir, tile
from concourse.bass_utils import run_bass_kernel_spmd

B, C, L, K = 32, 64, 16384, 3
LOUT = L - K + 1  # 16382
NCORES = 8
BPC = B // NCORES  # 4 batches per core
P = 128  # partitions (2 x C)
NJ = 512  # PSUM inner chunk (one fp32 bank; ISA max matmul free dim)
TH = L // 2  # 8192 deinterleaved columns
TOUT = LOUT // 2  # 8191 output column-pairs

F32 = mybir.dt.float32

# precision mode: f16 I/O (default, ~3e-4 rel err) or f32
MODE = os.environ.get("CONV_MODE", "f16")
CH = int(os.environ.get("CONV_CH", "2048"))  # T-cols per chunk (2x samples)
IBUFS = int(os.environ.get("CONV_IBUFS", "8"))
OBUFS = int(os.environ.get("CONV_OBUFS", "6"))
WARMUP = int(os.environ.get("CONV_WARMUP", "4"))
SYNC_TAIL = int(os.environ.get("CONV_SYNC_TAIL", "4"))
WQ = os.environ.get("CONV_WQ", "gpsimd")
RAMP = [int(v) for v in os.environ.get("CONV_RAMP", "512,1024").split(",") if v]
TAIL = [int(v) for v in os.environ.get("CONV_TAIL", "512,256").split(",") if v]

_NC_CACHE = []


def _io_dtypes():
    if MODE == "f16":
        return mybir.dt.float16, mybir.dt.float16, np.float16
    return F32, F32, np.float32


def _chunk_lists():
    """Per-batch T-column chunk lists. Batch 0 ramps up (DMA/PE clocks
    still ramping), the last batch ramps down (short tail)."""
    lists = {}
    for b in range(BPC):
        pre = RAMP if b == 0 else []
        post = TAIL if b == BPC - 1 else []
        rest = TOUT - sum(pre) - sum(post)
        body = [CH] * (rest // CH)
        last = rest - sum(body)
        lists[b] = pre + body + ([last] if last else []) + post
        assert sum(lists[b]) == TOUT and all(n > 0 for n in lists[b])
    return lists


def _build_nc():
    FIN, FOUT, _ = _io_dtypes()
    nc = bacc.Bacc("TRN2", target_bir_lowering=False, debug=False,
                   num_devices=NCORES)

    xd = nc.dram_tensor("xd", [BPC, P, TH], FIN, kind="ExternalInput")
    wT = nc.dram_tensor("wT", [P, 2, P], FIN, kind="ExternalInput")
    b2 = nc.dram_tensor("b2", [P, 1], F32, kind="ExternalInput")
    yd = nc.dram_tensor("yd", [BPC, P, TOUT], FOUT, kind="ExternalOutput")

    chunk_lists = _chunk_lists()
    nchunks = sum(len(v) for v in chunk_lists.values())

    with tile.TileContext(nc) as tc:
        with (
            tc.tile_pool(name="const", bufs=1) as const_pool,
            tc.tile_pool(name="inp", bufs=IBUFS) as inp_pool,
            tc.tile_pool(name="outp", bufs=OBUFS) as outp_pool,
            tc.tile_pool(name="psum", bufs=8, space=bass.MemorySpace.PSUM)
            as psum_pool,
        ):
            weng = nc.gpsimd if WQ == "gpsimd" else nc.sync
            w = const_pool.tile([P, 2, P], FIN)
            weng.dma_start(out=w[:], in_=wT[:])
            bias = const_pool.tile([P, 1], F32)
            weng.dma_start(out=bias[:], in_=b2[:])

            # HAM warm-up: dummy matmuls on zeroed SBUF while the first
            # input DMA is in flight, so clocks are ramped when real work
            # arrives. memset on GpSimd so this isn't gated on DVE start.
            if WARMUP:
                wz = const_pool.tile([P, 512], FIN)
                nc.gpsimd.memset(wz[:], 0.0)
                for i in range(WARMUP):
                    wp = psum_pool.tile([P, NJ], F32, tag="acc",
                                        name=f"warm{i}")
                    nc.tensor.matmul(wp[:], wz[:, :P], wz[:],
                                     start=True, stop=True)

            ci = 0
            for b in range(BPC):
                t0 = 0
                for nT in chunk_lists[b]:
                    # rhs needs one halo column (t0+nT+1 <= TOUT+1 <= TH)
                    it = inp_pool.tile([P, CH + 1], FIN, tag="in")
                    nc.sync.dma_start(out=it[:, :nT + 1],
                                      in_=xd[b, :, t0:t0 + nT + 1])
                    ot = outp_pool.tile([P, CH], FOUT, tag="out")
                    for j0 in range(0, nT, NJ):
                        nj = min(NJ, nT - j0)
                        pt = psum_pool.tile([P, NJ], F32, tag="acc")
                        nc.tensor.matmul(pt[:, :nj], w[:, 0, :],
                                         it[:, j0:j0 + nj],
                                         start=True, stop=False)
                        nc.tensor.matmul(pt[:, :nj], w[:, 1, :],
                                         it[:, j0 + 1:j0 + 1 + nj],
                                         start=False, stop=True)
                        # psum -> sbuf with fused bias add, split across
                        # ACT and DVE so the bank frees twice as fast
                        h = nj // 2
                        nc.scalar.add(ot[:, j0:j0 + h], pt[:, :h],
                                      add=bias[:, 0:1])
                        nc.vector.tensor_scalar_add(ot[:, j0 + h:j0 + nj],
                                                    pt[:, h:nj],
                                                    bias[:, 0:1])
                    oeng = nc.sync if ci >= nchunks - SYNC_TAIL else nc.gpsimd
                    oeng.dma_start(out=yd[b, :, t0:t0 + nT],
                                   in_=ot[:, :nT])
                    t0 += nT
                    ci += 1

    nc.compile()
    return nc


def _get_nc():
    if not _NC_CACHE:
        _NC_CACHE.append(_build_nc())
    return _NC_CACHE[0]


def _prep_weights(weight, bias, np_in):
    w0T, w1T, w2T = (np.ascontiguousarray(weight[:, :, k].T)
                     for k in range(K))
    A = np.zeros((P, P), np.float32)
    Bm = np.zeros((P, P), np.float32)
    A[0:C, 0:C] = w0T
    A[C:P, 0:C] = w1T
    A[C:P, C:P] = w0T
    Bm[0:C, 0:C] = w2T
    Bm[0:C, C:P] = w1T
    Bm[C:P, C:P] = w2T
    wT = np.stack([A, Bm], axis=1).astype(np_in)  # [P, 2, P]
    b2 = np.concatenate([bias, bias]).reshape(P, 1).astype(np.float32)
    return wT, b2


def kernel(x, weight, bias, _want_results=False, **run_kwargs):
    x = np.asarray(x, np.float32)
    weight = np.asarray(weight, np.float32)
    bias = np.asarray(bias, np.float32)
    _, _, np_in = _io_dtypes()
    nc = _get_nc()
    wT, b2 = _prep_weights(weight, bias, np_in)
    in_maps = []
    for i in range(NCORES):
        xs = x[BPC * i:BPC * (i + 1)]  # [BPC, C, L]
        # deinterleave: partition row e*64+ci holds x[ci, e::2]
        xdi = np.ascontiguousarray(
            xs.reshape(BPC, C, TH, 2).transpose(0, 3, 1, 2)
        ).reshape(BPC, P, TH).astype(np_in, copy=False)
        in_maps.append({"xd": xdi, "wT": wT, "b2": b2})
    res = run_bass_kernel_spmd(nc, in_maps, list(range(NCORES)), **run_kwargs)
    outs = []
    for i in range(NCORES):
        ydi = res.results[i]["yd"].astype(np.float32)  # [BPC, P, TOUT]
        # re-interleave: y[co, 2t+p] = yd[p*64+co, t]
        y = ydi.reshape(BPC, 2, C, TOUT).transpose(0, 2, 3, 1).reshape(
            BPC, C, LOUT)
        outs.append(y)
    out = np.ascontiguousarray(np.concatenate(outs, axis=0))
    if _want_results:
        return out, res
    return out
